# revision 1
# baseline (speedup 1.0000x reference)
"""Multi-head attention (B=4, N=2048, D=1024, H=16) on 8 TRN2 NeuronCores.

Sharding: 8 cores = batch(4) x sequence-half(2). Each core computes the full
attention output for its 1024-token slice of one batch (all 16 heads), so the
final unshard is a pure gather. The only cross-core traffic is an AllGather of
K^T and V between the two cores of each batch pair.

Per-core pipeline (bf16 matmul operands, fp32 PSUM accumulation):
  1. Cast x / w_qkv / w_proj to bf16, stage to DRAM, and DMA-transpose back so
     contraction dims sit on SBUF partitions.
  2. QKV projection. Q^T and K^T are produced in [d_out, token] orientation
     (lhsT = w_qkv^T tile, rhs = x^T); V in natural [token, d] orientation
     (lhsT = x^T tile, rhs = w_qkv^T).
  3. AllGather K^T then V across the pair (k-token axis spans both halves).
  4. Attention per head-pair p: S^T = (QK^T)^T via row-paired matmuls
     (contraction = head_dim 64, two heads in array row halves), exp on
     ScalarE straight out of PSUM (logits are bounded, no max subtraction),
     then O^T and the softmax denominator via col-paired matmuls over the
     k axis. The all-ones denominator lhsT replicates each head's denominator
     across its 64 output partitions, so normalization is a single full-width
     reciprocal + multiply on VectorE.
  5. Output projection from the accumulated attout^T tiles, bias add, DMA out.
"""

import sys

for _p in ("/opt/trn_rl_repo",):
    if _p not in sys.path:
        sys.path.insert(0, _p)

import numpy as np

import concourse.bass as bass
import concourse.mybir as mybir
import concourse.tile as tile
from concourse import bacc
from concourse.bass_utils import run_bass_kernel_spmd

B, N, D, H, HD = 4, 2048, 1024, 16, 64
SCALE = HD ** -0.5
NL = N // 2  # tokens per core
NCORES = 8
RG = [[0, 1], [2, 3], [4, 5], [6, 7]]
F32 = mybir.dt.float32
BF16 = mybir.dt.bfloat16
EXP = mybir.ActivationFunctionType.Exp


def _emit(tc, aps):
    nc = tc.nc
    x_l, wqkv, wproj, bias, out = (
        aps["x_local"], aps["w_qkv"], aps["w_proj"], aps["b_proj"], aps["out"])
    x_blk, wqkv_blk, wproj_blk = aps["x_blk"], aps["wqkv_blk"], aps["wproj_blk"]
    cc_k, cc_v, k_g, v_g = aps["cc_k"], aps["cc_v"], aps["k_g"], aps["v_g"]

    persist1 = tc.alloc_tile_pool(name="persist1", bufs=1)

    # ---- Phase A: load fp32, cast bf16, stage to DRAM column-blocked ------
    # (one [rows, 128] contiguous block per k-tile so the DMA-transposes
    # read contiguous DRAM). Loads on sync; fused blocked stores on scalar
    # (idle until the first exp). The sync queue carries ONLY prologue work
    # and transposes - every collective-gated DMA lives on scalar so the
    # in-order sync queue never blocks on a collective semaphore.
    prep = tc.alloc_tile_pool(name="prep", bufs=5)
    qkvp = tc.alloc_tile_pool(name="qkvp", bufs=1)

    def cast_tiles(src, blk, tiles):
        for i in tiles:
            t = prep.tile([128, D], F32, tag="ld_f32")
            nc.gpsimd.dma_start(out=t, in_=src[i * 128:(i + 1) * 128, :])
            tb = prep.tile([128, D], BF16, tag="cast_bf")
            nc.vector.tensor_copy(tb, t)
            dst = bass.AP(tensor=blk.tensor,
                          offset=blk.offset + i * 128 * 128,
                          ap=[[128, 128], [blk.ap[0][0], 8], [1, 128]])
            nc.scalar.dma_start(out=dst, in_=tb.rearrange("p (k c) -> p k c", k=8))

    # emission interleaves each group's loads with its transposes below

    bias_sb = persist1.tile([128, D], F32, tag="bias")
    bias_bcast = bass.AP(tensor=bias.tensor, offset=bias.offset,
                         ap=[[0, 128], *bias.ap])
    nc.scalar.dma_start(out=bias_sb, in_=bias_bcast)

    ones_sb = persist1.tile([128, 64], BF16, tag="ones")
    nc.vector.memset(ones_sb, 1.0)

    qT = [persist1.tile([128, NL], BF16, tag=f"qT{p}", name=f"qT{p}") for p in range(8)]
    kT = [persist1.tile([128, N], BF16, tag=f"kT{p}", name=f"kT{p}") for p in range(8)]
    vv = [persist1.tile([128, D], BF16, tag=f"v{kt}", name=f"v{kt}") for kt in range(16)]
    wpT_holder = [persist1.tile([128, D], BF16, tag=f"wpT{k}", name=f"wpT{k}")
                  for k in range(8)]


    xT = [qkvp.tile([128, NL], BF16, tag=f"xT{k}", name=f"xT{k}") for k in range(8)]
    wT = [qkvp.tile([128, 3 * D], BF16, tag=f"wT{k}", name=f"wT{k}") for k in range(8)]

    def wT_load(lo):
        for k in range(8):
            nc.sync.dma_start_transpose(
                out=wT[k][:, lo:lo + 1024], in_=wqkv_blk[k, lo:lo + 1024, :])

    # group-by-group: loads then the transposes that consume them, so the
    # in-order sync queue streams [loads | transposes] per group with no
    # cross-group blocking; all stores are on the scalar queue
    cast_tiles(x_l, x_blk, range(8))
    for k in range(8):
        nc.sync.dma_start_transpose(out=xT[k], in_=x_blk[k])
    cast_tiles(wqkv, wqkv_blk, range(8, 16))   # K rows 1024:2048
    wT_load(1024)
    cast_tiles(wqkv, wqkv_blk, range(16, 24))  # V rows 2048:3072
    wT_load(2048)
    cast_tiles(wqkv, wqkv_blk, range(0, 8))    # Q rows 0:1024
    wT_load(0)
    cast_tiles(wproj, wproj_blk, range(8))
    for k in range(8):
        nc.sync.dma_start_transpose(out=wpT_holder[k], in_=wproj_blk[k])

    with tc.tile_pool(name="qkvsb", bufs=2) as qkvsb, \
         tc.tile_pool(name="qkv_ps", bufs=2, space="PSUM") as qkvps:

        def proj_dT(m, dst_sb):
            ps = qkvps.tile([128, 2, 512], F32, tag="qkv_ps")
            for k in range(8):
                for qc in range(2):
                    nc.tensor.matmul(
                        out=ps[:, qc, :],
                        lhsT=wT[k][:, m * 128:(m + 1) * 128],
                        rhs=xT[k][:, qc * 512:(qc + 1) * 512],
                        start=(k == 0), stop=(k == 7))
            for qc in range(2):
                nc.vector.tensor_copy(dst_sb[:, qc * 512:(qc + 1) * 512], ps[:, qc, :])

        # K projection first so the K AllGather launches as early as possible
        for m in range(8, 16):
            ksb = qkvsb.tile([128, NL], BF16, tag="k_loc")
            proj_dT(m, ksb)
            nc.scalar.dma_start(out=cc_k[(m - 8) * 128:(m - 7) * 128, :], in_=ksb)
        nc.gpsimd.collective_compute(
            "AllGather", mybir.AluOpType.bypass, replica_groups=RG,
            ins=[cc_k], outs=[k_g])
        # gathered loads on scalar (its queue may block on the collective
        # semaphore without holding up any transpose)
        for p in range(8):
            nc.gpsimd.dma_start(out=kT[p][:, 0:NL], in_=k_g[0, p * 128:(p + 1) * 128, :])
            nc.gpsimd.dma_start(out=kT[p][:, NL:N], in_=k_g[1, p * 128:(p + 1) * 128, :])

        # V projection next so its AllGather overlaps the Q projection.
        # All remaining transposes are emitted BEFORE the ccV stores so the
        # in-order sync queue never parks a transpose behind a store that
        # waits on V-projection results.
        for t in range(8):
            vsb = qkvsb.tile([128, D], BF16, tag="v_loc")
            ps = qkvps.tile([128, 2, 512], F32, tag="qkv_ps")
            for k in range(8):
                for vc in range(2):
                    nc.tensor.matmul(
                        out=ps[:, vc, :],
                        lhsT=xT[k][:, t * 128:(t + 1) * 128],
                        rhs=wT[k][:, 2 * D + vc * 512:2 * D + (vc + 1) * 512],
                        start=(k == 0), stop=(k == 7))
            for vc in range(2):
                nc.vector.tensor_copy(vsb[:, vc * 512:(vc + 1) * 512], ps[:, vc, :])
            nc.scalar.dma_start(out=cc_v[t * 128:(t + 1) * 128, :], in_=vsb)
        nc.gpsimd.collective_compute(
            "AllGather", mybir.AluOpType.bypass, replica_groups=RG,
            ins=[cc_v], outs=[v_g])
        for kt in range(16):
            nc.gpsimd.dma_start(
                out=vv[kt], in_=v_g[kt // 8, (kt % 8) * 128:(kt % 8 + 1) * 128, :])

        # Q projection (overlaps the V gather; attention starts right after)
        for m in range(8):
            proj_dT(m, qT[m])

    qkvp.release()
    prep.release()

    # ---- Phase D: attention ----------------------------------------------
    persist2 = tc.alloc_tile_pool(name="persist2", bufs=1)
    attoutT = [persist2.tile([128, NL], BF16, tag=f"ao{p}", name=f"ao{p}") for p in range(8)]
    wpT = wpT_holder

    with tc.tile_pool(name="att_ps", bufs=2, space="PSUM") as attps, \
         tc.tile_pool(name="pT", bufs=4) as ppool, \
         tc.tile_pool(name="rc", bufs=2) as rpool:
        for p in range(8):
            for qc in range(2):
                o = attps.tile([128, 512], F32, tag="o_ps")
                dn = attps.tile([128, 512], F32, tag="den_ps")
                for kt in range(16):
                    s = attps.tile([128, 2, 512], F32, tag="s_ps")
                    for h in range(2):
                        nc.tensor.matmul(
                            out=s[:, h, :],
                            lhsT=kT[p][h * 64:(h + 1) * 64, kt * 128:(kt + 1) * 128],
                            rhs=qT[p][h * 64:(h + 1) * 64, qc * 512:(qc + 1) * 512],
                            start=True, stop=True,
                            tile_position=(h * 64, 0))
                    pt = ppool.tile([128, 2, 512], BF16, tag="pT")
                    nc.scalar.activation(pt, s, EXP, scale=SCALE)
                    for h in range(2):
                        nc.tensor.matmul(
                            out=o[h * 64:(h + 1) * 64, :],
                            lhsT=vv[kt][:, (2 * p + h) * 64:(2 * p + h + 1) * 64],
                            rhs=pt[:, h, :],
                            start=(kt == 0), stop=(kt == 15),
                            tile_position=(0, h * 64))
                    for h in range(2):
                        nc.tensor.matmul(
                            out=dn[h * 64:(h + 1) * 64, :],
                            lhsT=ones_sb,
                            rhs=pt[:, h, :],
                            start=(kt == 0), stop=(kt == 15),
                            tile_position=(0, h * 64))
                rc = rpool.tile([128, 512], F32, tag="rc")
                nc.vector.reciprocal(rc, dn)
                nc.vector.tensor_mul(attoutT[p][:, qc * 512:(qc + 1) * 512], o, rc)

    # ---- Phase E: output projection + bias --------------------------------
    with tc.tile_pool(name="proj_ps", bufs=2, space="PSUM") as projps, \
         tc.tile_pool(name="y_sb", bufs=3) as ypool:
        for tt in range(8):
            ps = projps.tile([128, 2, 512], F32, tag="proj_ps")
            for p in range(8):
                for ec in range(2):
                    nc.tensor.matmul(
                        out=ps[:, ec, :],
                        lhsT=attoutT[p][:, tt * 128:(tt + 1) * 128],
                        rhs=wpT[p][:, ec * 512:(ec + 1) * 512],
                        start=(p == 0), stop=(p == 7))
            yt = ypool.tile([128, D], F32, tag="y_sb")
            for ec in range(2):
                nc.vector.tensor_add(yt[:, ec * 512:(ec + 1) * 512], ps[:, ec, :],
                                     bias_sb[:, ec * 512:(ec + 1) * 512])
            nc.sync.dma_start(out=out[tt * 128:(tt + 1) * 128, :], in_=yt)
    persist2.release()
    persist1.release()


def _build():
    nc = bacc.Bacc("TRN2", target_bir_lowering=False, debug=False,
                   num_devices=NCORES)
    aps = {
        "x_local": nc.dram_tensor("x_local", [NL, D], F32, kind="ExternalInput").ap(),
        "w_qkv": nc.dram_tensor("w_qkv", [3 * D, D], F32, kind="ExternalInput").ap(),
        "w_proj": nc.dram_tensor("w_proj", [D, D], F32, kind="ExternalInput").ap(),
        "b_proj": nc.dram_tensor("b_proj", [D], F32, kind="ExternalInput").ap(),
        "out": nc.dram_tensor("out", [NL, D], F32, kind="ExternalOutput").ap(),
        "wqkv_blk": nc.dram_tensor("wqkv_blk", [8, 3 * D, 128], BF16).ap(),
        "wproj_blk": nc.dram_tensor("wproj_blk", [8, D, 128], BF16).ap(),
        "x_blk": nc.dram_tensor("x_blk", [8, NL, 128], BF16).ap(),
        "cc_k": nc.dram_tensor("cc_k", [D, NL], BF16).ap(),
        "cc_v": nc.dram_tensor("cc_v", [NL, D], BF16).ap(),
        "k_g": nc.dram_tensor("k_g", [2, D, NL], BF16).ap(),
        "v_g": nc.dram_tensor("v_g", [2, NL, D], BF16).ap(),
    }
    with tile.TileContext(nc) as tc:
        _emit(tc, aps)
    nc.compile()
    return nc


_NC = None


def _get_nc():
    global _NC
    if _NC is None:
        _NC = _build()
    return _NC


def run(x, w_qkv, w_proj, b_proj, **spmd_kwargs):
    nc = _get_nc()
    x = np.ascontiguousarray(np.asarray(x, dtype=np.float32))
    w_qkv = np.ascontiguousarray(np.asarray(w_qkv, dtype=np.float32))
    w_proj = np.ascontiguousarray(np.asarray(w_proj, dtype=np.float32))
    b_proj = np.ascontiguousarray(np.asarray(b_proj, dtype=np.float32))
    in_maps = []
    for c in range(NCORES):
        b, half = divmod(c, 2)
        in_maps.append({
            "x_local": np.ascontiguousarray(x[b, half * NL:(half + 1) * NL, :]),
            "w_qkv": w_qkv,
            "w_proj": w_proj,
            "b_proj": b_proj,
        })
    res = run_bass_kernel_spmd(nc, in_maps, list(range(NCORES)), **spmd_kwargs)
    y = np.empty((B, N, D), dtype=np.float32)
    for c in range(NCORES):
        b, half = divmod(c, 2)
        y[b, half * NL:(half + 1) * NL, :] = res.results[c]["out"]
    return y, res


def kernel(x, w_qkv, w_proj, b_proj):
    y, _ = run(x, w_qkv, w_proj, b_proj)
    return y



# revision 16
# speedup vs baseline: 1.2154x; 1.2154x over previous
"""Multi-head attention (B=4, N=2048, D=1024, H=16) on 8 TRN2 NeuronCores.

Sharding: 8 cores = batch(4) x sequence-half(2). Each core computes the full
attention output for its 1024-token slice of one batch (all 16 heads); the
only cross-core traffic is a pairwise AllGather of K^T and V.

Per-core pipeline (bf16 matmul operands, fp32 PSUM accumulation):
  1. Prologue: load x / w_qkv / w_proj fp32 in 128-row tiles (sync queue),
     cast bf16 on ScalarE (idle until attention), store CONTIGUOUS bf16 to
     DRAM (gpsimd queue), then ONE whole-block DMA-transpose per tensor
     block (sync queue, 3D out = [128, kblk, rows]) - 5 transposes total.
  2. K projection -> AllGather K^T. V projection -> AllGather V in four
     row-quarters so the first attention unit's O matmuls aren't gated on
     the full gather. Q projection m-tiles interleave with attention units.
  3. Attention per (head-pair p, q-half qc): S^T row-paired (contraction
     64), exp on ScalarE, O^T against a ones-AUGMENTED V (65th lhsT column
     = 1.0) so the softmax denominator accumulates in PSUM row 64 of the
     same chain - no denominator matmuls. k-tiles are visited in V-gather
     arrival order.
  4. Normalize: reciprocal of the [1,512] denominator row, DRAM-bounce
     broadcast to 64 partitions, one DVE multiply per head (partition-base
     mismatch between out and ins is fine on DVE).
  5. Output projection per qc half, interleaved into the other half's units.
"""

import sys

for _p in ("/opt/trn_rl_repo",):
    if _p not in sys.path:
        sys.path.insert(0, _p)

import numpy as np

import concourse.bass as bass
import concourse.mybir as mybir
import concourse.tile as tile
from concourse import bacc
from concourse.bass_utils import run_bass_kernel_spmd

B, N, D, H, HD = 4, 2048, 1024, 16, 64
SCALE = HD ** -0.5
NL = N // 2  # tokens per core
NCORES = 8
RG = [[0, 1], [2, 3], [4, 5], [6, 7]]
F32 = mybir.dt.float32
BF16 = mybir.dt.bfloat16
EXP = mybir.ActivationFunctionType.Exp
# k-tiles in V-gather quarter arrival order (quarter q covers local tokens
# 256q..256q+255 of both halves -> global k-tiles {2q, 2q+1, 2q+8, 2q+9})
KT_ORDER = [0, 1, 8, 9, 2, 3, 10, 11, 4, 5, 12, 13, 6, 7, 14, 15]


def _emit(tc, aps):
    nc = tc.nc
    x_l, wqkv, wproj, bias, out = (
        aps["x_local"], aps["w_qkv"], aps["w_proj"], aps["b_proj"], aps["out"])
    xbf, wqkvbf, wpbf = aps["xbf"], aps["wqkvbf"], aps["wpbf"]
    cc_k, cc_v, k_g, v_g = aps["cc_k"], aps["cc_v"], aps["k_g"], aps["v_g"]
    scratch = aps["scratch"]

    persist = tc.alloc_tile_pool(name="persist", bufs=1)

    bias_sb = persist.tile([128, D], F32, tag="bias")
    bias_bcast = bass.AP(tensor=bias.tensor, offset=bias.offset,
                         ap=[[0, 128], *bias.ap])
    nc.sync.dma_start(out=bias_sb, in_=bias_bcast)

    qTm = persist.tile([128, 8, NL], BF16, tag="qTm", name="qTm")
    kTm = persist.tile([128, 8, N], BF16, tag="kTm", name="kTm")
    wpTm = persist.tile([128, 8, D], BF16, tag="wpTm", name="wpTm")
    attoutT = persist.tile([128, 8, NL], BF16, tag="attoutT", name="attoutT")
    # ones-augmented V: per k-tile [128 ktok, 16 heads, 64 v + 1 ones]
    vv = [persist.tile([128, H, HD + 1], BF16, tag=f"vv{kt}", name=f"vv{kt}")
          for kt in range(16)]
    for kt in range(16):
        nc.gpsimd.memset(vv[kt][:, :, HD:HD + 1], 1.0)

    xt_pool = tc.alloc_tile_pool(name="xt", bufs=1)
    wt_pool = tc.alloc_tile_pool(name="wt", bufs=2)
    xTm = xt_pool.tile([128, 8, NL], BF16, tag="xTm", name="xTm")

    # transient prologue pools on the right stack so they release before the
    # attention pools allocate
    ld_pool = tc.alloc_tile_pool(name="ld", bufs=2, side="right")
    cast_pool = tc.alloc_tile_pool(name="cast", bufs=2, side="right")
    stage_pool = tc.alloc_tile_pool(name="stage", bufs=1, side="right")

    # K^T and V staging share one buffer: cc_k's store drains it before the
    # V projection copies land (WAR tracked by the pool).
    kstage = stage_pool.tile([128, 8, NL], BF16, tag="stage", name="kstage")

    def round_trip(src, dstbf, row0, nrows):
        # 128-row tiles: load fp32 (sync), cast (scalar), store bf16 (gpsimd)
        for c in range(nrows // 128):
            r0 = row0 + c * 128
            ld = ld_pool.tile([128, D], F32, tag="ld")
            nc.sync.dma_start(out=ld, in_=src[r0:r0 + 128, :])
            cb = cast_pool.tile([128, D], BF16, tag="cast")
            nc.scalar.copy(cb, ld)
            nc.gpsimd.dma_start(out=dstbf[r0:r0 + 128, :], in_=cb)

    # x, then w_qkv K rows, V rows, Q rows, then w_proj; a whole-block
    # DMA-transpose follows each block's round trip on the sync queue.
    round_trip(x_l, xbf, 0, NL)
    nc.sync.dma_start_transpose(out=xTm, in_=xbf)
    round_trip(wqkv, wqkvbf, D, D)             # K rows 1024:2048
    wtK = wt_pool.tile([128, 8, D], BF16, tag="wT", name="wtK")
    nc.sync.dma_start_transpose(out=wtK, in_=wqkvbf[D:2 * D, :])
    round_trip(wqkv, wqkvbf, 2 * D, D)         # V rows 2048:3072
    wtV = wt_pool.tile([128, 8, D], BF16, tag="wT", name="wtV")
    nc.sync.dma_start_transpose(out=wtV, in_=wqkvbf[2 * D:3 * D, :])
    round_trip(wqkv, wqkvbf, 0, D)             # Q rows 0:1024
    round_trip(wproj, wpbf, 0, D)
    nc.sync.dma_start_transpose(out=wpTm, in_=wpbf)

    # ---- Phase B: K/V projections + collectives -------------------------
    qkvps = tc.alloc_tile_pool(name="qkv_ps", bufs=2, space="PSUM")

    def proj_kv(wt, m, dst, as_lhsT):
        pss = []
        for qc in range(2):
            ps = qkvps.tile([128, 512], F32, tag="ps", name="ps")
            pss.append(ps)
        for k in range(8):
            for qc in range(2):
                if as_lhsT:  # V: out[tok, e]
                    nc.tensor.matmul(
                        out=pss[qc],
                        lhsT=xTm[:, k, m * 128:(m + 1) * 128],
                        rhs=wt[:, k, qc * 512:(qc + 1) * 512],
                        start=(k == 0), stop=(k == 7))
                else:        # K: out[e, tok]
                    nc.tensor.matmul(
                        out=pss[qc],
                        lhsT=wt[:, k, m * 128:(m + 1) * 128],
                        rhs=xTm[:, k, qc * 512:(qc + 1) * 512],
                        start=(k == 0), stop=(k == 7))
        for qc in range(2):
            nc.vector.tensor_copy(dst[:, qc * 512:(qc + 1) * 512], pss[qc])

    # K projection -> AllGather K^T -> kTm
    for m in range(8):
        proj_kv(wtK, m, kstage[:, m, :], as_lhsT=False)
    cck_dst = bass.AP(tensor=cc_k.tensor, offset=cc_k.offset,
                      ap=[[NL, 128], [128 * NL, 8], [1, NL]])
    nc.gpsimd.dma_start(out=cck_dst, in_=kstage)
    nc.gpsimd.collective_compute(
        "AllGather", mybir.AluOpType.bypass, replica_groups=RG,
        ins=[cc_k], outs=[k_g])
    for half in range(2):
        src = bass.AP(tensor=k_g.tensor, offset=k_g.offset + half * D * NL,
                      ap=[[NL, 128], [128 * NL, 8], [1, NL]])
        nc.sync.dma_start(out=kTm[:, :, half * NL:(half + 1) * NL], in_=src)

    # Q transpose rides the wt rotation (reuses wtK's buffer after K proj)
    wtQ = wt_pool.tile([128, 8, D], BF16, tag="wT", name="wtQ")
    nc.sync.dma_start_transpose(out=wtQ, in_=wqkvbf[0:D, :])

    # V projection -> AllGather V in four row-quarters -> vv tiles
    vstage = stage_pool.tile([128, 8, D], BF16, tag="stage", name="vstage")
    for q in range(4):
        for t in range(2 * q, 2 * q + 2):
            proj_kv(wtV, t, vstage[:, t, :], as_lhsT=True)
        ccv_dst = bass.AP(tensor=cc_v.tensor, offset=cc_v.offset + q * 256 * D,
                          ap=[[D, 128], [128 * D, 2], [1, D]])
        nc.gpsimd.dma_start(out=ccv_dst, in_=vstage[:, 2 * q:2 * q + 2, :])
        nc.gpsimd.collective_compute(
            "AllGather", mybir.AluOpType.bypass, replica_groups=RG,
            ins=[cc_v[q * 256:(q + 1) * 256, :]],
            outs=[v_g[q]])
        for half in range(2):
            for tl in range(2):
                kt = half * 8 + 2 * q + tl
                nc.sync.dma_start(
                    out=vv[kt][:, :, 0:HD],
                    in_=v_g[q, half, tl * 128:(tl + 1) * 128, :]
                        .rearrange("p (h c) -> p h c", h=H))

    qkvps.release()
    stage_pool.release()
    cast_pool.release()
    ld_pool.release()

    # ---- Phase C: Q projection interleaved with attention ---------------
    spool = tc.alloc_tile_pool(name="s_ps", bufs=2, space="PSUM")
    oapool = tc.alloc_tile_pool(name="oa_ps", bufs=2, space="PSUM")
    ptpool = tc.alloc_tile_pool(name="pt", bufs=6)
    rcpool = tc.alloc_tile_pool(name="rc", bufs=2)
    rbcpool = tc.alloc_tile_pool(name="rbc", bufs=2)
    ytpool = tc.alloc_tile_pool(name="yt", bufs=2)

    def proj_q(m):
        ps = spool.tile([128, 2, 512], F32, tag="s", name="ps_q")
        for k in range(8):
            for qc in range(2):
                nc.tensor.matmul(
                    out=ps[:, qc, :],
                    lhsT=wtQ[:, k, m * 128:(m + 1) * 128],
                    rhs=xTm[:, k, qc * 512:(qc + 1) * 512],
                    start=(k == 0), stop=(k == 7))
        nc.vector.tensor_copy(qTm[:, m, :], ps.rearrange("p a b -> p (a b)"))

    def unit(p, qc):
        oa = [oapool.tile([128, 512], F32, tag=f"oa{h}", name=f"oa{h}")
              for h in range(2)]
        for i, kt in enumerate(KT_ORDER):
            s = spool.tile([128, 2, 512], F32, tag="s", name="s")
            for h in range(2):
                nc.tensor.matmul(
                    out=s[:, h, :],
                    lhsT=kTm[h * 64:(h + 1) * 64, p, kt * 128:(kt + 1) * 128],
                    rhs=qTm[h * 64:(h + 1) * 64, p, qc * 512:(qc + 1) * 512],
                    start=True, stop=True,
                    tile_position=(h * 64, 0))
            pt = ptpool.tile([128, 2, 512], BF16, tag="pt", name="pt")
            nc.scalar.activation(pt, s, EXP, scale=SCALE)
            for h in range(2):
                nc.tensor.matmul(
                    out=oa[h][0:HD + 1, :],
                    lhsT=vv[kt][:, 2 * p + h, :],
                    rhs=pt[:, h, :],
                    start=(i == 0), stop=(i == 15))
        # normalize: reciprocal of the denominator row (PSUM row 64), DRAM
        # bounce to broadcast across 64 partitions, one multiply per head
        u = qc * 8 + p
        rc = rcpool.tile([1, 2, 512], F32, tag="rc", name="rc")
        for h in range(2):
            nc.vector.reciprocal(rc[:, h, :], oa[h][HD:HD + 1, :])
        nc.gpsimd.dma_start(out=scratch[u], in_=rc)
        rbc = rbcpool.tile([64, 2, 512], F32, tag="rbc", name="rbc")
        rsrc = bass.AP(tensor=scratch.tensor,
                       offset=scratch.offset + u * 1024,
                       ap=[[0, 64], [512, 2], [1, 512]])
        nc.sync.dma_start(out=rbc, in_=rsrc)
        for h in range(2):
            nc.vector.tensor_mul(
                attoutT[h * 64:(h + 1) * 64, p, qc * 512:(qc + 1) * 512],
                oa[h][0:HD, :], rbc[:, h, :])

    def outproj(tt):
        yt = ytpool.tile([128, D], F32, tag="yt", name="yt")
        ps = spool.tile([128, 2, 512], F32, tag="s", name="ps_o")
        for p in range(8):
            for ec in range(2):
                nc.tensor.matmul(
                    out=ps[:, ec, :],
                    lhsT=attoutT[:, p, tt * 128:(tt + 1) * 128],
                    rhs=wpTm[:, p, ec * 512:(ec + 1) * 512],
                    start=(p == 0), stop=(p == 7))
        for ec in range(2):
            nc.vector.tensor_add(yt[:, ec * 512:(ec + 1) * 512], ps[:, ec, :],
                                 bias_sb[:, ec * 512:(ec + 1) * 512])
        nc.sync.dma_start(out=out[tt * 128:(tt + 1) * 128, :], in_=yt)

    # qc0 pass: Q projection m-tiles lead their consuming unit by one
    proj_q(0)
    proj_q(1)
    for p in range(8):
        if p >= 2:
            proj_q(p)
        unit(p, 0)
    # outproj(qc0 half) emitted two units into the qc1 pass so the PE never
    # waits on the last qc0 unit's normalize round trip
    for p in range(8):
        unit(p, 1)
        if p == 1:
            for tt in range(4):
                outproj(tt)
    for tt in range(4, 8):
        outproj(tt)

    ytpool.release()
    rbcpool.release()
    rcpool.release()
    ptpool.release()
    oapool.release()
    spool.release()
    wt_pool.release()
    xt_pool.release()
    persist.release()


def _build():
    nc = bacc.Bacc("TRN2", target_bir_lowering=False, debug=False,
                   num_devices=NCORES)
    aps = {
        "x_local": nc.dram_tensor("x_local", [NL, D], F32, kind="ExternalInput").ap(),
        "w_qkv": nc.dram_tensor("w_qkv", [3 * D, D], F32, kind="ExternalInput").ap(),
        "w_proj": nc.dram_tensor("w_proj", [D, D], F32, kind="ExternalInput").ap(),
        "b_proj": nc.dram_tensor("b_proj", [D], F32, kind="ExternalInput").ap(),
        "out": nc.dram_tensor("out", [NL, D], F32, kind="ExternalOutput").ap(),
        "xbf": nc.dram_tensor("xbf", [NL, D], BF16).ap(),
        "wqkvbf": nc.dram_tensor("wqkvbf", [3 * D, D], BF16).ap(),
        "wpbf": nc.dram_tensor("wpbf", [D, D], BF16).ap(),
        "cc_k": nc.dram_tensor("cc_k", [D, NL], BF16).ap(),
        "cc_v": nc.dram_tensor("cc_v", [NL, D], BF16).ap(),
        "k_g": nc.dram_tensor("k_g", [2, D, NL], BF16).ap(),
        "v_g": nc.dram_tensor("v_g", [4, 2, 256, D], BF16).ap(),
        "scratch": nc.dram_tensor("scratch", [16, 2, 512], F32).ap(),
    }
    with tile.TileContext(nc) as tc:
        _emit(tc, aps)
    nc.compile()
    return nc


_NC = None


def _get_nc():
    global _NC
    if _NC is None:
        _NC = _build()
    return _NC


def run(x, w_qkv, w_proj, b_proj, **spmd_kwargs):
    nc = _get_nc()
    x = np.ascontiguousarray(np.asarray(x, dtype=np.float32))
    w_qkv = np.ascontiguousarray(np.asarray(w_qkv, dtype=np.float32))
    w_proj = np.ascontiguousarray(np.asarray(w_proj, dtype=np.float32))
    b_proj = np.ascontiguousarray(np.asarray(b_proj, dtype=np.float32))
    in_maps = []
    for c in range(NCORES):
        b, half = divmod(c, 2)
        in_maps.append({
            "x_local": np.ascontiguousarray(x[b, half * NL:(half + 1) * NL, :]),
            "w_qkv": w_qkv,
            "w_proj": w_proj,
            "b_proj": b_proj,
        })
    res = run_bass_kernel_spmd(nc, in_maps, list(range(NCORES)), **spmd_kwargs)
    y = np.empty((B, N, D), dtype=np.float32)
    for c in range(NCORES):
        b, half = divmod(c, 2)
        y[b, half * NL:(half + 1) * NL, :] = res.results[c]["out"]
    return y, res


def kernel(x, w_qkv, w_proj, b_proj):
    y, _ = run(x, w_qkv, w_proj, b_proj)
    return y


# revision 27
# speedup vs baseline: 1.4341x; 1.1799x over previous
"""Multi-head attention (B=4, N=2048, D=1024, H=16) on 8 TRN2 NeuronCores.

Sharding: 8 cores = batch(4) x sequence-half(2). Each core computes the full
attention output for its 1024-token slice of one batch (all 16 heads); the
only cross-core traffic is a pairwise AllGather of K^T and V.

Per-core pipeline (bf16 matmul operands, fp32 PSUM accumulation):
  1. Prologue: fp32 loads stream on TWO queues (x on scalar, weights on
     sync) at ~150GB/s; casts to bf16 on ScalarE; transposes ON THE PE
     (identity matmul, bf16, PSUM->SBUF copies on Pool) - no DRAM staging
     round trip. Only w_proj keeps a DMA-transpose round trip, executed in
     the background during attention.
  2. K projection -> AllGather K^T. V projection -> AllGather V in four
     row-quarters so the first attention unit's O matmuls aren't gated on
     the full gather. Q projection m-tiles interleave with attention units
     (their PSUM->SBUF copies run on Pool so DVE recips can't block them).
  3. Attention per (head-pair p, q-half qc): S^T row-paired (contraction
     64), exp on ScalarE, O^T against a ones-AUGMENTED V (65th lhsT column
     = 1.0) so the softmax denominator accumulates in PSUM row 64 of the
     same chain - no denominator matmuls. k-tiles visit in V-gather
     arrival order.
  4. Normalize: denominator rows copied (Pool) into one tile, single DVE
     reciprocal, DRAM-bounce broadcast to 64 partitions, one DVE multiply
     per head (partition-base mismatch between out and ins is fine).
  5. Output projection per qc half, interleaved into the other half's units.
"""

import sys

for _p in ("/opt/trn_rl_repo",):
    if _p not in sys.path:
        sys.path.insert(0, _p)

import numpy as np

import concourse.bass as bass
import concourse.masks as masks
import concourse.mybir as mybir
import concourse.tile as tile
from concourse import bacc
from concourse.bass_utils import run_bass_kernel_spmd

B, N, D, H, HD = 4, 2048, 1024, 16, 64
SCALE = HD ** -0.5
NL = N // 2  # tokens per core
NCORES = 8
RG = [[0, 1], [2, 3], [4, 5], [6, 7]]
F32 = mybir.dt.float32
BF16 = mybir.dt.bfloat16
EXP = mybir.ActivationFunctionType.Exp
VW = 66  # vv row pitch (64 v + 1 ones + 1 pad for 4B alignment)
# k-tiles in V-gather quarter arrival order (quarter q covers local tokens
# 256q..256q+255 of both halves -> global k-tiles {2q, 2q+1, 2q+8, 2q+9})
KT_ORDER = [0, 1, 8, 9, 2, 3, 10, 11, 4, 5, 12, 13, 6, 7, 14, 15]


def _emit(tc, aps):
    nc = tc.nc
    x_l, wqkv, wproj, bias, out = (
        aps["x_local"], aps["w_qkv"], aps["w_proj"], aps["b_proj"], aps["out"])
    wpbf = aps["wpbf"]
    cc_k, cc_v, k_g, v_g = aps["cc_k"], aps["cc_v"], aps["k_g"], aps["v_g"]
    scratch = aps["scratch"]

    persist = tc.alloc_tile_pool(name="persist", bufs=1)

    bias_sb = persist.tile([128, D], F32, tag="bias")
    bias_bcast = bass.AP(tensor=bias.tensor, offset=bias.offset,
                         ap=[[0, 128], *bias.ap])
    nc.sync.dma_start(out=bias_sb, in_=bias_bcast)

    ident = persist.tile([128, 128], BF16, tag="ident", name="ident")
    masks.make_identity(nc, ident[:])

    qTm = persist.tile([128, 8, NL], BF16, tag="qTm", name="qTm")
    kTm = persist.tile([128, 8, N], BF16, tag="kTm", name="kTm")
    wpTm = persist.tile([128, 8, D], BF16, tag="wpTm", name="wpTm")
    attoutT = persist.tile([128, 8, NL], BF16, tag="attoutT", name="attoutT")
    # ones-augmented V: per k-tile [128 ktok, 16 heads, 64 v | 1 ones | pad]
    vv = [persist.tile([128, H, VW], BF16, tag=f"vv{kt}", name=f"vv{kt}")
          for kt in range(16)]
    for kt in range(16):
        nc.gpsimd.memset(vv[kt][:, :, HD:HD + 1], 1.0)

    xt_pool = tc.alloc_tile_pool(name="xt", bufs=1)
    wt_pool = tc.alloc_tile_pool(name="wt", bufs=2)
    xTm = xt_pool.tile([128, 8, NL], BF16, tag="xTm", name="xTm")

    # transient prologue pools on the right stack so they release before the
    # attention pools allocate
    ld_pool = tc.alloc_tile_pool(name="ld", bufs=5, side="right")
    cast_pool = tc.alloc_tile_pool(name="cast", bufs=3, side="right")
    stage_pool = tc.alloc_tile_pool(name="stage", bufs=1, side="right")

    # K^T and V staging share one buffer: cc_k's store drains it before the
    # V projection copies land (WAR tracked by the pool).
    kstage = stage_pool.tile([128, 8, NL], BF16, tag="stage", name="kstage")

    # PSUM: ps (K/V proj) + tr (PE transposes) tags, 2 bufs each = 4 banks
    qkvps = tc.alloc_tile_pool(name="qkv_ps", bufs=2, space="PSUM")

    def load128(src, row0, ldq):
        # 128-row fp32 chunk on queue ldq
        ld = ld_pool.tile([128, D], F32, tag="ld", name="ld")
        ldq.dma_start(out=ld, in_=src[row0:row0 + 128, :])
        return ld

    def cast128(ld):
        cb = cast_pool.tile([128, D], BF16, tag="cast", name="cb")
        nc.scalar.copy(cb, ld)
        return cb

    def pe_transpose(cb, dstm, r0):
        # transpose a [128, D] bf16 row-tile into dstm[:, :, r0:r0+128] via
        # identity matmuls; one Pool copy moves PSUM -> SBUF
        trp = qkvps.tile([128, 8, 128], BF16, tag="tr", name="trp")
        for c in range(8):
            nc.tensor.matmul(
                out=trp[:, c, :],
                lhsT=cb[:, c * 128:(c + 1) * 128],
                rhs=ident,
                is_transpose=True)
        nc.vector.tensor_copy(dstm[:, :, r0:r0 + 128], trp)

    # x (scalar queue) and w_qkv K rows (sync queue) load concurrently with
    # interleaved buffer rotation; casts follow arrival order on ScalarE.
    # V and Q row loads are emitted behind them on sync; their casts and PE
    # transposes are deferred into the K/V projection phases below.
    x_cb, wk_cb = [], []
    for c in range(8):
        xl = load128(x_l, c * 128, nc.scalar)
        kl = load128(wqkv, D + c * 128, nc.sync)
        x_cb.append(cast128(xl))
        wk_cb.append(cast128(kl))
    wv_ld = [load128(wqkv, 2 * D + c * 128, nc.sync) for c in range(8)]
    wq_ld = [load128(wqkv, c * 128, nc.sync) for c in range(8)]
    # transposes interleave x/wK in the same order as the casts so the cast
    # pool's slot rotation never inverts against the in-order PE queue
    wtK = wt_pool.tile([128, 8, D], BF16, tag="wT", name="wtK")
    for c in range(8):
        pe_transpose(x_cb[c], xTm, c * 128)
        pe_transpose(wk_cb[c], wtK, c * 128)

    # ---- Phase B: K/V projections + collectives -------------------------
    def proj_kv(wt, m, dst, as_lhsT):
        pss = []
        for qc in range(2):
            ps = qkvps.tile([128, 512], F32, tag="ps", name="ps")
            pss.append(ps)
        for k in range(8):
            for qc in range(2):
                if as_lhsT:  # V: out[tok, e]
                    nc.tensor.matmul(
                        out=pss[qc],
                        lhsT=xTm[:, k, m * 128:(m + 1) * 128],
                        rhs=wt[:, k, qc * 512:(qc + 1) * 512],
                        start=(k == 0), stop=(k == 7))
                else:        # K: out[e, tok]
                    nc.tensor.matmul(
                        out=pss[qc],
                        lhsT=wt[:, k, m * 128:(m + 1) * 128],
                        rhs=xTm[:, k, qc * 512:(qc + 1) * 512],
                        start=(k == 0), stop=(k == 7))
        for qc in range(2):
            nc.vector.tensor_copy(dst[:, qc * 512:(qc + 1) * 512], pss[qc])

    # K projection; wV casts+transposes interleave after m=5
    wtV = wt_pool.tile([128, 8, D], BF16, tag="wT", name="wtV")
    for m in range(8):
        proj_kv(wtK, m, kstage[:, m, :], as_lhsT=False)
        if m == 5:
            for c in range(8):
                pe_transpose(cast128(wv_ld[c]), wtV, c * 128)
    cck_dst = bass.AP(tensor=cc_k.tensor, offset=cc_k.offset,
                      ap=[[NL, 128], [128 * NL, 8], [1, NL]])
    nc.gpsimd.dma_start(out=cck_dst, in_=kstage)
    nc.gpsimd.collective_compute(
        "AllGather", mybir.AluOpType.bypass, replica_groups=RG,
        ins=[cc_k], outs=[k_g])
    for half in range(2):
        src = bass.AP(tensor=k_g.tensor, offset=k_g.offset + half * D * NL,
                      ap=[[NL, 128], [128 * NL, 8], [1, NL]])
        nc.sync.dma_start(out=kTm[:, :, half * NL:(half + 1) * NL], in_=src)

    # V projection in four row-quarters; wQ transposes interleave after q=1
    wtQ = wt_pool.tile([128, 8, D], BF16, tag="wT", name="wtQ")
    vstage = stage_pool.tile([128, 8, D], BF16, tag="stage", name="vstage")
    for q in range(4):
        for t in range(2 * q, 2 * q + 2):
            proj_kv(wtV, t, vstage[:, t, :], as_lhsT=True)
        if q == 1:
            for c in range(8):
                pe_transpose(cast128(wq_ld[c]), wtQ, c * 128)
        ccv_dst = bass.AP(tensor=cc_v.tensor, offset=cc_v.offset + q * 256 * D,
                          ap=[[D, 128], [128 * D, 2], [1, D]])
        nc.gpsimd.dma_start(out=ccv_dst, in_=vstage[:, 2 * q:2 * q + 2, :])
        nc.gpsimd.collective_compute(
            "AllGather", mybir.AluOpType.bypass, replica_groups=RG,
            ins=[cc_v[q * 256:(q + 1) * 256, :]],
            outs=[v_g[q]])
        for half in range(2):
            for tl in range(2):
                kt = half * 8 + 2 * q + tl
                nc.sync.dma_start(
                    out=vv[kt][:, :, 0:HD],
                    in_=v_g[q, half, tl * 128:(tl + 1) * 128, :]
                        .rearrange("p (h c) -> p h c", h=H))

    # w_proj round trip (DMA-transpose) runs in the background during
    # attention: loads on sync, casts on DVE, stores gpsimd, transpose sync
    for c in range(8):
        ld = ld_pool.tile([128, D], F32, tag="ld", name="ld")
        nc.sync.dma_start(out=ld, in_=wproj[c * 128:(c + 1) * 128, :])
        cb = cast_pool.tile([128, D], BF16, tag="cast", name="cb")
        nc.vector.tensor_copy(cb, ld)
        nc.gpsimd.dma_start(out=wpbf[c * 128:(c + 1) * 128, :], in_=cb)
    nc.sync.dma_start_transpose(out=wpTm, in_=wpbf)

    qkvps.release()
    stage_pool.release()
    cast_pool.release()
    ld_pool.release()

    # ---- Phase C: Q projection interleaved with attention ---------------
    spool = tc.alloc_tile_pool(name="s_ps", bufs=2, space="PSUM")
    oapool = tc.alloc_tile_pool(name="oa_ps", bufs=2, space="PSUM")
    ptpool = tc.alloc_tile_pool(name="pt", bufs=6)
    rcpool = tc.alloc_tile_pool(name="rc", bufs=2)
    rbcpool = tc.alloc_tile_pool(name="rbc", bufs=2)
    ytpool = tc.alloc_tile_pool(name="yt", bufs=1)

    def proj_q(m):
        ps = spool.tile([128, 2, 512], F32, tag="s", name="ps_q")
        for k in range(8):
            for qc in range(2):
                nc.tensor.matmul(
                    out=ps[:, qc, :],
                    lhsT=wtQ[:, k, m * 128:(m + 1) * 128],
                    rhs=xTm[:, k, qc * 512:(qc + 1) * 512],
                    start=(k == 0), stop=(k == 7))
        nc.vector.tensor_copy(qTm[:, m, :], ps.rearrange("p a b -> p (a b)"))

    def unit(p, qc):
        oa = [oapool.tile([128, 512], F32, tag=f"oa{h}", name=f"oa{h}")
              for h in range(2)]
        for i, kt in enumerate(KT_ORDER):
            s = spool.tile([128, 2, 512], F32, tag="s", name="s")
            for h in range(2):
                nc.tensor.matmul(
                    out=s[:, h, :],
                    lhsT=kTm[h * 64:(h + 1) * 64, p, kt * 128:(kt + 1) * 128],
                    rhs=qTm[h * 64:(h + 1) * 64, p, qc * 512:(qc + 1) * 512],
                    start=True, stop=True,
                    tile_position=(h * 64, 0))
            pt = ptpool.tile([128, 2, 512], BF16, tag="pt", name="pt")
            nc.scalar.activation(pt, s, EXP, scale=SCALE)
            for h in range(2):
                nc.tensor.matmul(
                    out=oa[h][0:HD + 1, :],
                    lhsT=vv[kt][:, 2 * p + h, 0:HD + 1],
                    rhs=pt[:, h, :],
                    start=(i == 0), stop=(i == 15))
        # normalize: denominator rows (PSUM row 64) -> one tile (Pool), one
        # DVE reciprocal, DRAM bounce to broadcast across 64 partitions,
        # then one multiply per head
        u = qc * 8 + p
        rc = rcpool.tile([1, 2, 512], F32, tag="rc", name="rc")
        dn = rcpool.tile([1, 2, 512], F32, tag="dn", name="dn")
        for h in range(2):
            nc.scalar.copy(dn[:, h, :], oa[h][HD:HD + 1, :])
        nc.vector.reciprocal(rc, dn)
        nc.gpsimd.dma_start(out=scratch[u], in_=rc)
        rbc = rbcpool.tile([64, 2, 512], F32, tag="rbc", name="rbc")
        rsrc = bass.AP(tensor=scratch.tensor,
                       offset=scratch.offset + u * 1024,
                       ap=[[0, 64], [512, 2], [1, 512]])
        nc.sync.dma_start(out=rbc, in_=rsrc)
        for h in range(2):
            nc.vector.tensor_mul(
                attoutT[h * 64:(h + 1) * 64, p, qc * 512:(qc + 1) * 512],
                oa[h][0:HD, :], rbc[:, h, :])

    def outproj(tt):
        yt = ytpool.tile([128, D], F32, tag="yt", name="yt")
        ps = spool.tile([128, 2, 512], F32, tag="s", name="ps_o")
        for p in range(8):
            for ec in range(2):
                nc.tensor.matmul(
                    out=ps[:, ec, :],
                    lhsT=attoutT[:, p, tt * 128:(tt + 1) * 128],
                    rhs=wpTm[:, p, ec * 512:(ec + 1) * 512],
                    start=(p == 0), stop=(p == 7))
        for ec in range(2):
            nc.vector.tensor_add(yt[:, ec * 512:(ec + 1) * 512], ps[:, ec, :],
                                 bias_sb[:, ec * 512:(ec + 1) * 512])
        nc.sync.dma_start(out=out[tt * 128:(tt + 1) * 128, :], in_=yt)

    # qc0 pass: Q projection m-tiles lead their consuming unit by two
    proj_q(0)
    proj_q(1)
    for p in range(8):
        if p + 2 < 8:
            proj_q(p + 2)
        unit(p, 0)
    # outproj(qc0 half) emitted two units into the qc1 pass so the PE never
    # waits on the last qc0 unit's normalize round trip
    for p in range(8):
        unit(p, 1)
        if p == 1:
            for tt in range(4):
                outproj(tt)
    for tt in range(4, 8):
        outproj(tt)

    ytpool.release()
    rbcpool.release()
    rcpool.release()
    ptpool.release()
    oapool.release()
    spool.release()
    wt_pool.release()
    xt_pool.release()
    persist.release()


def _build():
    nc = bacc.Bacc("TRN2", target_bir_lowering=False, debug=False,
                   num_devices=NCORES)
    aps = {
        "x_local": nc.dram_tensor("x_local", [NL, D], F32, kind="ExternalInput").ap(),
        "w_qkv": nc.dram_tensor("w_qkv", [3 * D, D], F32, kind="ExternalInput").ap(),
        "w_proj": nc.dram_tensor("w_proj", [D, D], F32, kind="ExternalInput").ap(),
        "b_proj": nc.dram_tensor("b_proj", [D], F32, kind="ExternalInput").ap(),
        "out": nc.dram_tensor("out", [NL, D], F32, kind="ExternalOutput").ap(),
        "wpbf": nc.dram_tensor("wpbf", [D, D], BF16).ap(),
        "cc_k": nc.dram_tensor("cc_k", [D, NL], BF16).ap(),
        "cc_v": nc.dram_tensor("cc_v", [NL, D], BF16).ap(),
        "k_g": nc.dram_tensor("k_g", [2, D, NL], BF16).ap(),
        "v_g": nc.dram_tensor("v_g", [4, 2, 256, D], BF16).ap(),
        "scratch": nc.dram_tensor("scratch", [16, 2, 512], F32).ap(),
    }
    with tile.TileContext(nc) as tc:
        _emit(tc, aps)
    nc.compile()
    return nc


_NC = None


def _get_nc():
    global _NC
    if _NC is None:
        _NC = _build()
    return _NC


def run(x, w_qkv, w_proj, b_proj, **spmd_kwargs):
    nc = _get_nc()
    x = np.ascontiguousarray(np.asarray(x, dtype=np.float32))
    w_qkv = np.ascontiguousarray(np.asarray(w_qkv, dtype=np.float32))
    w_proj = np.ascontiguousarray(np.asarray(w_proj, dtype=np.float32))
    b_proj = np.ascontiguousarray(np.asarray(b_proj, dtype=np.float32))
    in_maps = []
    for c in range(NCORES):
        b, half = divmod(c, 2)
        in_maps.append({
            "x_local": np.ascontiguousarray(x[b, half * NL:(half + 1) * NL, :]),
            "w_qkv": w_qkv,
            "w_proj": w_proj,
            "b_proj": b_proj,
        })
    res = run_bass_kernel_spmd(nc, in_maps, list(range(NCORES)), **spmd_kwargs)
    y = np.empty((B, N, D), dtype=np.float32)
    for c in range(NCORES):
        b, half = divmod(c, 2)
        y[b, half * NL:(half + 1) * NL, :] = res.results[c]["out"]
    return y, res


def kernel(x, w_qkv, w_proj, b_proj):
    y, _ = run(x, w_qkv, w_proj, b_proj)
    return y
